# revision 1
# baseline (speedup 1.0000x reference)
"""AttentionBlock (GroupNorm + MHA + proj + residual) on 8 Trainium2 cores.

Sharding: data-parallel over batch (b=8, one sample per NeuronCore).
Per-core kernel computes the full block for one sample entirely on-chip:

  x [512, 1024] -> GroupNorm(32 groups) -> qkv (fp32r matmuls)
    -> per-head QK^T (K=64, two heads packed into PE row groups)
    -> exp on ScalarE (softmax denominator via an extra ones column in the
       AV matmul's stationary operand)
    -> AV (K=128) -> normalize -> proj + bias + residual -> out [512, 1024]

Host-side preprocessing folds norm_w/norm_b into the qkv weights, folds the
1/8 attention scale into the Q weights, and pre-permutes/transposes all
weights so the device never transposes anything. V is produced directly in
transposed [s, c] layout by swapping the matmul operands.
"""
import sys

sys.path.insert(0, "/opt/trn_rl_repo")

import numpy as np

import concourse.bacc as bacc
import concourse.mybir as mybir
from concourse.bass_utils import run_bass_kernel_spmd
from concourse.tile import TileContext

AF = mybir.ActivationFunctionType
OP = mybir.AluOpType
F32 = mybir.dt.float32
F32R = mybir.dt.float32r
BF16 = mybir.dt.bfloat16

B, C, HH, WW = 8, 512, 32, 32
L = HH * WW          # 1024
H = 8                # heads
HD = C // H          # 64
G = 32               # groups
GSZ = C // G         # 16 channels per group
EPS = 1e-5
N_CORES = 8
EXP_BUFS = 34

_CACHE = {}


def _build_module():
    if "nc" in _CACHE:
        return _CACHE["nc"]
    nc = bacc.Bacc("TRN2", target_bir_lowering=False, debug=False)

    x_d = nc.dram_tensor("x", [C, L], F32, kind="ExternalInput")
    xb_d = nc.dram_tensor("xb", [C, L], BF16, kind="ExternalInput")
    wqk_d = nc.dram_tensor("wqk", [C, 2 * C], BF16, kind="ExternalInput")
    bqk_d = nc.dram_tensor("bqk", [128, 8], F32, kind="ExternalInput")
    wv_d = nc.dram_tensor("wv", [C, C], BF16, kind="ExternalInput")
    bvb_d = nc.dram_tensor("bvb", [1, C], F32, kind="ExternalInput")
    wp_d = nc.dram_tensor("wp", [C, C], BF16, kind="ExternalInput")
    pb_d = nc.dram_tensor("pb", [128, 4], F32, kind="ExternalInput")
    gfw_d = nc.dram_tensor("gfw", [128, 128], F32, kind="ExternalInput")
    gbw_d = nc.dram_tensor("gbw", [G, C], F32, kind="ExternalInput")
    ones8_d = nc.dram_tensor("ones8", [128, 8], BF16, kind="ExternalInput")
    out_d = nc.dram_tensor("out", [C, L], F32, kind="ExternalOutput")

    with TileContext(nc) as tc:
        with tc.tile_pool(name="persist", bufs=1) as per, \
             tc.tile_pool(name="expp", bufs=EXP_BUFS) as expp, \
             tc.tile_pool(name="outp", bufs=3) as outp, \
             tc.tile_pool(name="small", bufs=2) as smallp, \
             tc.tile_pool(name="acc", bufs=2, space="PSUM") as accp, \
             tc.tile_pool(name="sps", bufs=3, space="PSUM") as spp:

            # ---------- persistent tiles + input DMAs ----------
            xt = [per.tile([128, L], F32, tag=f"xt{j}", name=f"xt{j}") for j in range(4)]
            xbt = [per.tile([128, L], BF16, tag=f"xb{j}", name=f"xb{j}") for j in range(4)]
            for hf in range(2):
                for j in range(4):
                    nc.sync.dma_start(
                        out=xbt[j][:, 512 * hf:512 * hf + 512],
                        in_=xb_d[128 * j:128 * j + 128, 512 * hf:512 * hf + 512])
            gfw_t = per.tile([128, 128], F32, tag="gfw", name="gfw")
            nc.sync.dma_start(out=gfw_t[:, :], in_=gfw_d[:, :])
            dmy = per.tile([1, 1], F32, tag="dmy", name="dmy")
            nc.scalar.activation(out=dmy[:, :], in_=gfw_t[0:1, 0:1], func=AF.Exp)
            wup = accp.tile([128, 128], F32, tag="acc", name="acc")
            for _ in range(14):
                nc.tensor.matmul(wup[:, :], gfw_t[:, :], gfw_t[:, :],
                                 start=True, stop=True)
            wqk = [per.tile([128, 2 * C], BF16, tag=f"wqk{k}", name=f"wqk{k}") for k in range(4)]
            for k in range(4):
                nc.sync.dma_start(out=wqk[k][:, :], in_=wqk_d[128 * k:128 * k + 128, :])
            gbw_t = per.tile([G, C], F32, tag="gbw", name="gbw")
            nc.sync.dma_start(out=gbw_t[:, :], in_=gbw_d[:, :])
            bqk_t = per.tile([128, 8], F32, tag="bqk", name="bqk")
            nc.sync.dma_start(out=bqk_t[:, :], in_=bqk_d[:, :])
            wv = [per.tile([128, C], BF16, tag=f"wv{k}", name=f"wv{k}") for k in range(4)]
            for k in range(4):
                nc.sync.dma_start(out=wv[k][:, :], in_=wv_d[128 * k:128 * k + 128, :])
            ones8_t = per.tile([128, 8], BF16, tag="ones8", name="ones8")
            nc.sync.dma_start(out=ones8_t[:, :], in_=ones8_d[:, :])
            bvr_t = per.tile([1, C], F32, tag="bvr", name="bvr")
            nc.sync.dma_start(out=bvr_t[:, :], in_=bvb_d[:, :])
            bvb_t = per.tile([128, C], F32, tag="bvb", name="bvb")
            nc.gpsimd.partition_broadcast(bvb_t[:, :], bvr_t[:, :], channels=128)
            for j in range(4):
                nc.sync.dma_start(out=xt[j][:, :], in_=x_d[128 * j:128 * j + 128, :])
            wp = [per.tile([128, C], BF16, tag=f"wp{k}", name=f"wp{k}") for k in range(4)]
            pb_t = per.tile([128, 4], F32, tag="pb", name="pb")

            xn = [per.tile([128, L], BF16, tag=f"xn{j}", name=f"xn{j}") for j in range(4)]
            a_t = [per.tile([128, L], BF16, tag=f"a{j}", name=f"a{j}") for j in range(4)]
            qp = [per.tile([128, L], BF16, tag=f"qp{j}", name=f"qp{j}") for j in range(4)]
            kp = [per.tile([128, L], BF16, tag=f"kp{j}", name=f"kp{j}") for j in range(4)]
            vt = [per.tile([128, H * (HD + 1)], BF16, tag=f"vt{j}", name=f"vt{j}") for j in range(8)]

            # ---------- GroupNorm ----------
            # per-channel [sum_h0, sum_h1, sumsq_h0, sumsq_h1]; halves start
            # as soon as each half-tile DMA lands.
            stats = [per.tile([128, 4], F32, tag=f"st{j}", name=f"st{j}") for j in range(4)]
            for j in range(4):
                for hf in range(2):
                    sl = slice(512 * hf, 512 * hf + 512)
                    nc.scalar.activation(out=xn[j][:, sl], in_=xbt[j][:, sl],
                                         func=AF.Copy,
                                         accum_out=stats[j][:, hf:hf + 1])
                    nc.vector.scalar_tensor_tensor(out=a_t[j][:, sl],
                                                   in0=xbt[j][:, sl],
                                                   scalar=1.0, in1=xbt[j][:, sl],
                                                   op0=OP.mult, op1=OP.mult,
                                                   accum_out=stats[j][:, 2 + hf:3 + hf])
            gst = accp.tile([G, 4], F32, tag="acc", name="acc")
            for j in range(4):
                nc.tensor.matmul(gst[:, :], gfw_t[:, 32 * j:32 * j + 32],
                                 stats[j][:, :], start=(j == 0), stop=(j == 3))
            gsum = per.tile([G, 2], F32, tag="gsum", name="gsum")   # [Sx, Sxx]
            msb = per.tile([G, 2], F32, tag="msb", name="msb")      # [mean | E[x^2]]
            msq = per.tile([G, 1], F32, tag="msq", name="msq")
            veps = per.tile([G, 1], F32, tag="veps", name="veps")
            sstd = per.tile([G, 1], F32, tag="sstd", name="sstd")
            gsb = per.tile([G, 2], F32, tag="gsb", name="gsb")     # [rstd | -mean*rstd]
            gst_sb = per.tile([G, 4], F32, tag="gst_sb", name="gst_sb")
            nc.vector.tensor_copy(gst_sb[:, :], gst[:, :])
            nc.vector.tensor_tensor(out=gsum[:, :], in0=gst_sb[:, 0:4:2],
                                    in1=gst_sb[:, 1:4:2], op=OP.add)
            nc.vector.tensor_scalar(out=msb[:, :], in0=gsum[:, :],
                                    scalar1=1.0 / (GSZ * L), scalar2=None,
                                    op0=OP.mult)
            nc.vector.tensor_tensor(out=msq[:, :], in0=msb[:, 0:1],
                                    in1=msb[:, 0:1], op=OP.mult)
            nc.vector.scalar_tensor_tensor(out=veps[:, :], in0=msb[:, 1:2],
                                           scalar=EPS, in1=msq[:, :],
                                           op0=OP.add, op1=OP.subtract)
            nc.scalar.activation(out=sstd[:, :], in_=veps[:, :], func=AF.Ln)
            nc.scalar.activation(out=gsb[:, 0:1], in_=sstd[:, :], func=AF.Exp,
                                 scale=-0.5)
            nc.vector.scalar_tensor_tensor(out=gsb[:, 1:2], in0=msb[:, 0:1],
                                           scalar=-1.0, in1=gsb[:, 0:1],
                                           op0=OP.mult, op1=OP.mult)
            cb = [per.tile([128, 2], F32, tag=f"cb{j}", name=f"cb{j}") for j in range(4)]
            for j in range(4):
                cbp = accp.tile([128, 2], F32, tag="acc", name="acc")
                nc.tensor.matmul(cbp[:, :], gbw_t[:, 128 * j:128 * j + 128],
                                 gsb[:, :], start=True, stop=True)
                nc.vector.tensor_copy(cb[j][:, :], cbp[:, :])
                nc.scalar.activation(out=xn[j][:, :], in_=xbt[j][:, :],
                                     func=AF.Identity,
                                     bias=cb[j][:, 1:2], scale=cb[j][:, 0:1])

            # ---------- helpers ----------
            class QkvStream:
                """qkv output chunks m (each 8 matmuls + a bias copy) as an
                emit-on-demand stream of individual matmuls."""
                def __init__(self, ms):
                    self.jobs = [(m, n2) for m in ms for n2 in range(2)]
                    self.i = 0
                    self.pq = None

                def emit(self, k):
                    for _ in range(k):
                        if self.i >= 8 * len(self.jobs) // 2:
                            return
                        job, kc = divmod(self.i, 4)
                        m, n2 = self.jobs[job]
                        if kc == 0:
                            self.pq = accp.tile([128, 512], F32, tag="acc",
                                                name="acc")
                        nc.tensor.matmul(self.pq[:, :],
                                         wqk[kc][:, 128 * m:128 * m + 128],
                                         xn[kc][:, 512 * n2:512 * n2 + 512],
                                         start=(kc == 0), stop=(kc == 3))
                        if kc == 3:
                            dest = qp[m] if m < 4 else kp[m - 4]
                            nc.vector.tensor_scalar(
                                out=dest[:, 512 * n2:512 * n2 + 512],
                                in0=self.pq[:, :],
                                scalar1=bqk_t[:, m:m + 1], scalar2=None,
                                op0=OP.add)
                        self.i += 1

            def qkv_chunk(m):
                QkvStream([m]).emit(8)

            def vt_chunk(sc):
                """v^T for s-chunk sc, all heads: [128 s, 8*(64+1)] layout with
                a ones column per head (accumulates the softmax denominator)."""
                pv = accp.tile([128, 512], F32, tag="acc", name="acc")
                for kc in range(4):
                    nc.tensor.matmul(pv[:, :],
                                     xn[kc][:, 128 * sc:128 * sc + 128],
                                     wv[kc][:, :], start=(kc == 0), stop=(kc == 3))
                v3 = vt[sc][:, :].rearrange("p (h e) -> p h e", e=HD + 1)
                nc.vector.tensor_copy(vt[sc][:, HD::HD + 1], ones8_t[:, :])
                nc.vector.tensor_tensor(
                    out=v3[:, :, 0:HD],
                    in0=pv[:, :].rearrange("p (h e) -> p h e", e=HD),
                    in1=bvb_t[:, :].rearrange("p (h e) -> p h e", e=HD),
                    op=OP.add)

            def norm_head(p, e, n2, pa, act_copy=False):
                """softmax-normalize one AV accumulator into a_t. The PSUM
                accumulator is drained immediately (denominator row + raw
                numerator) so the slot recycles fast; the normalization then
                runs SBUF-side in place."""
                base = 64 * e
                asl = a_t[p][base:base + 64, 512 * n2:512 * n2 + 512]
                dsb = smallp.tile([1, 512], F32, tag="dsb", name="dsb")
                if act_copy:
                    nc.scalar.copy(dsb[:, :], pa[HD:HD + 1, :])
                    nc.vector.tensor_copy(asl, pa[0:HD, :])
                else:
                    nc.vector.tensor_copy(dsb[:, :], pa[HD:HD + 1, :])
                    nc.scalar.copy(asl, pa[0:HD, :])
                db = smallp.tile([128, 512], F32, tag="db", name="db")
                nc.gpsimd.partition_broadcast(db[:, :], dsb[:, :], channels=128)
                rb = smallp.tile([128, 512], F32, tag="rb", name="rb")
                nc.vector.reciprocal_approx_fast(out=rb[:, :], in_=db[:, :])
                nc.vector.tensor_tensor(out=asl, in0=asl,
                                        in1=rb[base:base + 64, :], op=OP.mult)

            def attn_A(p, prev=None, qkv=None, stream_vt=False, own_av=None):
                """S^T + exp for pair p; pair p-1's AV matmuls and pair p+1's
                qkv matmuls ride along per chunk, emitted ahead of the S
                matmuls so the strict-FIFO PE never idles behind an S matmul
                waiting for a free S-psum slot."""
                est = [[None] * 8, [None] * 8]
                if own_av is not None:
                    own_av.est = est
                av = AvStream(prev) if prev is not None else None
                for sc in range(8):
                    if av is not None:
                        av.emit(4)
                    if qkv is not None:
                        qkv.emit(2)
                    if stream_vt:
                        vt_chunk(sc)
                    if own_av is not None and sc >= 1:
                        own_av.emit(1)
                    for e in range(2):
                        base = 64 * e
                        ps_s = spp.tile([128, L], F32, tag="sps", name="sps")
                        for n2 in range(2):
                            nc.tensor.matmul(
                                ps_s[:, 512 * n2:512 * n2 + 512],
                                kp[p][base:base + 64, 128 * sc:128 * sc + 128],
                                qp[p][base:base + 64, 512 * n2:512 * n2 + 512],
                                start=True, stop=True, tile_position=(base, 0))
                        es = expp.tile([128, L], BF16, tag="expS", name="expS")
                        nc.scalar.activation(out=es[:, :], in_=ps_s[:, :],
                                             func=AF.Exp)
                        est[e][sc] = es
                return est

            class AvStream:
                """AV accumulation sweeps as an emit-on-demand stream
                (8 matmuls per sweep; norm emitted when a sweep closes).
                One PSUM accumulator live at a time."""
                def __init__(self, pe, sweeps=None, act_copy=False):
                    self.p, self.est = pe
                    self.sweeps = sweeps or [(0, 0), (1, 0), (0, 1), (1, 1)]
                    self.act_copy = act_copy
                    self.i = 0
                    self.pa = None

                def emit(self, k):
                    for _ in range(k):
                        if self.i >= 8 * len(self.sweeps):
                            return
                        sweep, sc = divmod(self.i, 8)
                        e, n2 = self.sweeps[sweep]
                        h = 2 * self.p + e
                        if sc == 0:
                            self.pa = accp.tile([HD + 1, 512], F32,
                                                tag="acc", name="acc")
                        nc.tensor.matmul(
                            self.pa[:, :], vt[sc][:, 65 * h:65 * h + 65],
                            self.est[e][sc][:, 512 * n2:512 * n2 + 512],
                            start=(sc == 0), stop=(sc == 7))
                        if sc == 7:
                            norm_head(self.p, e, n2, self.pa,
                                      act_copy=self.act_copy)
                        self.i += 1

            # ---------- emission schedule ----------
            # qkv pair 0 first; v^T chunks stream inside pair 0's loop;
            # pair p-1's AV sweeps and pair p+1's qkv chunks ride inside
            # pair p's chunk loop.
            qkv_chunk(0)
            qkv_chunk(4)
            prev = None
            own3 = None
            for p in range(4):
                qs = QkvStream([p + 1, p + 5]) if p + 1 < 4 else None
                if p == 3:
                    own3 = AvStream((3, None), sweeps=[(0, 0)])
                    est_cur = attn_A(p, prev, qs, own_av=own3)
                else:
                    est_cur = attn_A(p, prev, qs, stream_vt=(p == 0))
                if qs is not None:
                    qs.emit(16)  # drain any remainder
                prev = (p, est_cur)
            # proj weights arrive late on purpose (not needed until the tail)
            for k in range(4):
                nc.sync.dma_start(out=wp[k][:, :], in_=wp_d[128 * k:128 * k + 128, :])
            nc.sync.dma_start(out=pb_t[:, :], in_=pb_d[:, :])

            class ProjStream:
                """proj groups (m, n2): 4 accumulating matmuls then fused
                bias+residual and the output DMA."""
                def __init__(self, n2):
                    self.n2 = n2
                    self.i = 0
                    self.po = None

                def emit(self, k):
                    for _ in range(k):
                        if self.i >= 16:
                            return
                        m, cc = divmod(self.i, 4)
                        n2 = self.n2
                        if cc == 0:
                            self.po = spp.tile([128, 512], F32, tag="sps",
                                               name="sps")
                        nc.tensor.matmul(self.po[:, :],
                                         wp[cc][:, 128 * m:128 * m + 128],
                                         a_t[cc][:, 512 * n2:512 * n2 + 512],
                                         start=(cc == 0), stop=(cc == 3))
                        if cc == 3:
                            ob = outp.tile([128, 512], F32, tag="ob", name="ob")
                            nc.vector.scalar_tensor_tensor(
                                out=ob[:, :], in0=self.po[:, :],
                                scalar=pb_t[:, m:m + 1],
                                in1=xt[m][:, 512 * n2:512 * n2 + 512],
                                op0=OP.add, op1=OP.add)
                            nc.sync.dma_start(
                                out=out_d[128 * m:128 * m + 128,
                                          512 * n2:512 * n2 + 512],
                                in_=ob[:, :])
                        self.i += 1

            # tail: finish pair 3's AV sweeps overlapped with proj halves
            own3.emit(8)                      # drain (0,0) remainder
            av10 = AvStream(prev, sweeps=[(1, 0)], act_copy=True)
            av10.emit(8)
            avn1 = AvStream(prev, sweeps=[(0, 1), (1, 1)], act_copy=True)
            pr0, pr1 = ProjStream(0), ProjStream(1)
            for _ in range(4):
                avn1.emit(4)
                pr0.emit(4)
            pr1.emit(16)

    nc.compile()
    _CACHE["nc"] = nc
    return nc


def _prep_constants(norm_w, norm_b, qkv_w, qkv_b, proj_w, proj_b):
    norm_w = np.asarray(norm_w, np.float64)
    norm_b = np.asarray(norm_b, np.float64)
    qkv_w = np.asarray(qkv_w, np.float64)
    qkv_b = np.asarray(qkv_b, np.float64)
    proj_w = np.asarray(proj_w, np.float64)
    proj_b = np.asarray(proj_b, np.float64)

    idx = np.arange(HD)
    q_idx = np.concatenate([h * 3 * HD + idx for h in range(H)])
    k_idx = q_idx + HD
    v_idx = q_idx + 2 * HD

    # fold norm affine: qkv = W @ (gn*nw + nb) + b = (W*nw) @ gn + (W@nb + b)
    Wf = qkv_w * norm_w[None, :]
    bf = qkv_b + qkv_w @ norm_b
    s2 = 1.0 / np.sqrt(HD)  # both q*scale and k*scale -> fold s^2 into q
    Wq, bq = Wf[q_idx] * s2, bf[q_idx] * s2
    Wk, bk = Wf[k_idx], bf[k_idx]
    Wv, bv = Wf[v_idx], bf[v_idx]

    wqk = np.concatenate([Wq.T, Wk.T], axis=1)                  # [512, 1024]
    bqk = np.concatenate([bq, bk]).reshape(8, 128).T            # [128, 8]
    wv = np.ascontiguousarray(Wv.T)                             # [512, 512]
    bvb = np.tile(bv[None, :], (128, 1))                        # [128, 512]
    wp = np.ascontiguousarray(proj_w.T)                         # [512, 512]
    pb = proj_b.reshape(4, 128).T                               # [128, 4]

    # gfw column block j (used as lhsT [128, 32] for channel chunk j): maps
    # channel 128j+p to its global group 8j + p//16.
    ch = np.arange(C)
    gfw = np.zeros((128, 128), np.float64)
    for j in range(4):
        for p_ in range(128):
            gfw[p_, 32 * j + 8 * j + p_ // GSZ] = 1.0
    gbw = (ch[None, :] // GSZ == np.arange(G)[:, None]).astype(np.float64)

    import ml_dtypes
    f = np.float32
    bf = ml_dtypes.bfloat16
    return dict(ones8=np.ones((128, 8), bf),
                wqk=np.ascontiguousarray(wqk.astype(bf)),
                bqk=np.ascontiguousarray(bqk, f),
                wv=np.ascontiguousarray(wv.astype(bf)),
                bvb=np.ascontiguousarray(bv[None, :], f),
                wp=np.ascontiguousarray(wp.astype(bf)),
                pb=np.ascontiguousarray(pb, f), gfw=np.ascontiguousarray(gfw, f),
                gbw=np.ascontiguousarray(gbw, f))


def kernel(x, norm_w, norm_b, qkv_w, qkv_b, proj_w, proj_b, _trace=False):
    x = np.asarray(x, np.float32)
    consts = _prep_constants(norm_w, norm_b, qkv_w, qkv_b, proj_w, proj_b)
    nc = _build_module()
    in_maps = []
    for i in range(N_CORES):
        xi = np.ascontiguousarray(x[i].reshape(C, L))
        import ml_dtypes as _md
        m = {"x": xi, "xb": np.ascontiguousarray(xi.astype(_md.bfloat16))}
        m.update(consts)
        in_maps.append(m)
    res = run_bass_kernel_spmd(nc, in_maps, core_ids=list(range(N_CORES)),
                               trace=_trace)
    out = np.stack([res.results[i]["out"] for i in range(N_CORES)])
    if _trace:
        _CACHE["last_results"] = res
    return out.reshape(B, C, HH, WW).astype(np.float32)

